# revision 14
# baseline (speedup 1.0000x reference)
"""Trainium2 Bass kernel for nn_AdaptiveLiquidLayer (RK4 liquid-neuron layer).

Computation (per batch row b, neuron n):
    ic   = x @ W_in^T                      # input current, shared by RK4 stages
    ode(s) = -s/tau + sigmoid(sigma*(ic + w*s + bias)) * (A - s),  w = w_rec*mask
    RK4 with DT=1:  out = h + (k1 + 2k2 + 2k3 + k4)/6

Strategy:
  - Pure data parallel over batch across 8 NeuronCores (8192 rows/core).
  - Batch-on-partition layout: tiles [128 batch, 256 neurons].
  - Neurons permuted so "unmasked" (sigma*w_rec*mask != 0) come first.
    For masked neurons the ODE is linear in the state (sigmoid argument is
    state-independent), so the whole RK4 update collapses to
        out = h + (A*f - (f+u)*h) * R(f),   f = sigmoid(sigma*ic + sigma*b)
    with R a cubic polynomial (coefficients computed on host) -> 6 DVE
    tensor_tensor passes instead of the full 4-stage chain.
  - All 2-tensor ops are fp16 tensor_tensor on DVE (2x perf mode);
    scalar-affine ops go to ScalarE activations (engine balance) or DVE
    tensor_scalar (4x). scalar_tensor_tensor is avoided (1x only).
  - fp16 on-chip + fp16 HBM I/O (PSUM accumulates fp32); per-neuron scalar
    params are uniform for this problem and are baked as immediates.
"""

import os
import sys
import types
from contextlib import ExitStack

import numpy as np

for _p in ("/opt/trn_rl_repo", "/opt/pypackages"):
    if os.path.isdir(_p) and _p not in sys.path:
        sys.path.append(_p)

import concourse.bass as bass  # noqa: E402
import concourse.tile as tile  # noqa: E402
import concourse.tile_utils as _tu  # noqa: E402

_tu.max_sbuf_usage = 204 * 1024  # cayman has 208K usable; default 192K is stale
from concourse import bacc, mybir  # noqa: E402
from concourse.bass_utils import run_bass_kernel_spmd  # noqa: E402

Op = mybir.AluOpType
Act = mybir.ActivationFunctionType
F16 = mybir.dt.float16
F32 = mybir.dt.float32

N_CORES = 8
B, I, N = 65536, 128, 256
BS = B // N_CORES  # 8192 rows per core
P = 128            # partitions (batch-tile rows)
T = BS // P        # 64 batch tiles per core
DT = 1.0

G = 32             # batch tiles per elementwise group
NG = T // G        # groups
PSG = 8            # batch tiles per PSUM tile (4 banks)
PSBUFS = 2         # psum pool bufs
MSUB = 16          # batch tiles per masked-path sub-chain
USUB = 1           # independent unmasked chains per group
SCHUNK = 1         # ScalarE emission chunks inside unmasked chain
UBUFS = 10
MBUFS = 8
FIRST_CHUNKS = 4   # extra DMA chunking for group 0 ramp

LAST_EXEC_TIME_NS = None
LAST_RESULT = None


def _install_ntff_hook():
    """Register the axon NTFF profiling hook so trace=True works."""
    if "antenv.axon_hooks" in sys.modules:
        return
    try:
        import antenv
        from trn_agent_boot.trn_boot import _ntff_profile_via_ctypes

        mod = types.ModuleType("antenv.axon_hooks")
        _h = {}
        mod.set_axon_ntff_profile_hook = lambda hook: _h.__setitem__("h", hook)
        mod.get_axon_ntff_profile_hook = lambda: _h.get("h")
        sys.modules["antenv.axon_hooks"] = mod
        antenv.axon_hooks = mod
        mod.set_axon_ntff_profile_hook(
            _ntff_profile_via_ctypes("/opt/axon/libaxon_pjrt.so")
        )
    except Exception:
        pass


def _uniform(arr, name):
    a = np.asarray(arr, dtype=np.float32)
    v = float(a.reshape(-1)[0])
    if not np.all(a == v):
        raise NotImplementedError(f"non-uniform {name} not supported")
    return v


def _v3(ap, n):
    return ap.rearrange("p (t n) -> p t n", n=n)


def _build(nu, nm, sig_v, sb_v, u_v, A_v, rc):
    """Build the 8-core SPMD program. rc = cubic coeffs [c3, c2, c1, c0] of
    R(f) = DT*P(DT*(f+u))/6 for the masked closed-form path."""
    nc = bacc.Bacc("TRN2", target_bir_lowering=False, debug=False,
                   num_devices=N_CORES)

    x_d = nc.dram_tensor("x", [P, BS], F16, kind="ExternalInput").ap()
    h_d = nc.dram_tensor("h", [P, T * N], F16, kind="ExternalInput").ap()
    w_d = nc.dram_tensor("w", [P, N], F16, kind="ExternalInput").ap()
    sw_d = (nc.dram_tensor("sw", [P, G * nu], F16, kind="ExternalInput").ap()
            if nu else None)
    out_d = nc.dram_tensor("out", [P, T * N], F16, kind="ExternalOutput").ap()

    c3, c2, c1, c0 = (float(v) for v in rc)
    ctr = iter(range(100000))

    def scal_act(dst, src, func=Act.Copy, scale=1.0, bias=0.0, chunks=1):
        fd = dst.shape[-1]
        step = fd // chunks
        for i in range(chunks):
            sl = slice(i * step, (i + 1) * step)
            nc.scalar.activation(dst[:, sl], src[:, sl], func,
                                 bias=bias, scale=scale)

    def vec_ts(dst, src, s1, s2, op0, op1=None):
        if s2 is None:
            nc.vector.tensor_scalar(dst, src, s1, None, op0)
        else:
            nc.vector.tensor_scalar(dst, src, s1, s2, op0, op1)

    with tile.TileContext(nc) as tc, ExitStack() as ctx:
        const = ctx.enter_context(tc.tile_pool(name="const", bufs=1))
        psum = ctx.enter_context(
            tc.tile_pool(name="psum", bufs=PSBUFS, space="PSUM"))
        evac = ctx.enter_context(tc.tile_pool(name="evac", bufs=2))
        utmp = ctx.enter_context(tc.tile_pool(name="utmp", bufs=UBUFS))
        mtmp = ctx.enter_context(tc.tile_pool(name="mtmp", bufs=MBUFS))
        outp = ctx.enter_context(tc.tile_pool(name="outp", bufs=2))

        x_sb = const.tile([P, BS], F16)
        h_sb = const.tile([P, T * N], F16)
        w_sb = const.tile([P, N], F16)
        nc.sync.dma_start(w_sb[:], w_d[:])
        if nu:
            sw_sb = const.tile([P, G * nu], F16)
            nc.sync.dma_start(sw_sb[:], sw_d[:])

        for g in range(NG):
            hg = _v3(h_sb[:, g * G * N:(g + 1) * G * N], N)
            out_t = outp.tile([P, G * N], F16, name=f"out_{g}", tag="out")
            og = _v3(out_t[:], N)

            s_m = (evac.tile([P, G * nm], F16, name=f"s_m_{g}", tag="s_m")
                   if nm else None)
            z0 = (evac.tile([P, G * nu], F16, name=f"z0_{g}", tag="z0")
                  if nu else None)

            # ---- DMA in (chunked per PSUM sub-group), matmul, evacuation ----
            pgs = [(0, 2), (2, 8), (8, 16), (16, 24), (24, 32)] if g == 0 \
                else [(0, 8), (8, 16), (16, 24), (24, 32)]
            for (t0, t1) in pgs:
                xsl = slice((g * G + t0) * P, (g * G + t1) * P)
                nc.sync.dma_start(x_sb[:, xsl], x_d[:, xsl])
                hsl = slice((g * G + t0) * N, (g * G + t1) * N)
                nc.gpsimd.dma_start(h_sb[:, hsl], h_d[:, hsl])
                nt = t1 - t0
                ps = psum.tile([P, PSG * N], F32, name=f"ps_{g}_{t0}",
                               tag="ps")
                for j in range(nt):
                    ti = g * G + t0 + j
                    nc.tensor.matmul(
                        ps[:, j * N:(j + 1) * N],
                        x_sb[:, ti * P:(ti + 1) * P],
                        w_sb[:],
                        start=True, stop=True,
                    )
                ps3 = _v3(ps[:, :nt * N], N)
                if nm:
                    dst = _v3(s_m[:, t0 * nm:t1 * nm], nm)
                    nc.scalar.activation(dst, ps3[:, :, nu:N], Act.Sigmoid,
                                         bias=sb_v, scale=sig_v)
                if nu:
                    dst = _v3(z0[:, t0 * nu:t1 * nu], nu)
                    nc.scalar.activation(dst, ps3[:, :, 0:nu], Act.Copy,
                                         bias=sb_v, scale=sig_v)

            # ---- masked columns: closed-form cubic path (Estrin) ----
            def masked_sub(t0, t1):
                FDm = (t1 - t0) * nm
                s_q = s_m[:, t0 * nm:t1 * nm]
                h_q = hg[:, t0:t1, nu:N]
                o_q = og[:, t0:t1, nu:N]

                def mt():
                    return mtmp.tile([P, FDm], F16, name=f"mt_{next(ctr)}",
                                     tag="mtmp")

                # R(s) = (c3 s + c2) s^2 + (c1 s + c0)   (Estrin)
                s2 = mt()
                scal_act(s2[:], s_q, Act.Square, chunks=2)
                ra = mt()
                vec_ts(ra[:], s_q, c3, c2, Op.mult, Op.add)
                rb = mt()
                vec_ts(rb[:], s_q, c1, c0, Op.mult, Op.add)
                rt = mt()
                nc.vector.tensor_tensor(rt[:], ra[:], s2[:], Op.mult)
                rr = mt()
                nc.vector.tensor_tensor(rr[:], rt[:], rb[:], Op.add)
                # k1 = A*s - (s+u)*h ; out = h + R*k1
                su = mt()
                scal_act(su[:], s_q, bias=u_v, chunks=2)
                m2 = mt()
                nc.vector.tensor_tensor(_v3(m2[:], nm), _v3(su[:], nm),
                                        h_q, Op.mult)
                k1 = mt()
                if A_v == 1.0:
                    nc.vector.tensor_tensor(k1[:], s_q, m2[:], Op.subtract)
                else:
                    sA = mt()
                    vec_ts(sA[:], s_q, A_v, None, Op.mult)
                    nc.vector.tensor_tensor(k1[:], sA[:], m2[:], Op.subtract)
                gg = mt()
                nc.vector.tensor_tensor(gg[:], rr[:], k1[:], Op.mult)
                nc.vector.tensor_tensor(o_q, h_q, _v3(gg[:], nm), Op.add)

            if not nm:
                msubs = []
            elif g == 0:
                msubs = [(0, 2), (2, 8), (8, 16), (16, 24), (24, 32)]
            else:
                msubs = [(0, 8), (8, 16), (16, 24), (24, 32)]

            def next_masked():
                if msubs:
                    masked_sub(*msubs.pop(0))

            if nm and g == 0:
                next_masked()
                next_masked()

            # ---- unmasked columns: 4-stage RK4 chain ----
            if nu:
                FD = G * nu
                h_u = hg[:, :, 0:nu]
                o_u = og[:, :, 0:nu]

                def ut():
                    return utmp.tile([P, FD], F16, name=f"ut_{next(ctr)}",
                                     tag="utmp")

                # stage 1 (state = h)
                m = ut()
                nc.vector.tensor_tensor(_v3(m[:], nu), _v3(sw_sb[:], nu), h_u,
                                        Op.mult)
                z = ut()
                nc.vector.tensor_tensor(z[:], z0[:], m[:], Op.add)
                f = ut()
                scal_act(f[:], z[:], Act.Sigmoid, chunks=SCHUNK)
                fu = ut()
                scal_act(fu[:], f[:], bias=u_v, chunks=SCHUNK)
                p = ut()
                nc.vector.tensor_tensor(_v3(p[:], nu), _v3(fu[:], nu), h_u,
                                        Op.mult)
                k_prev = ut()
                nc.vector.tensor_tensor(k_prev[:], f[:], p[:], Op.subtract)
                acc = k_prev
                # stages 2..4
                for st, c in ((2, DT * 0.5), (3, DT * 0.5), (4, DT)):
                    s_j = ut()
                    if c == 1.0:
                        nc.vector.tensor_tensor(_v3(s_j[:], nu),
                                                _v3(k_prev[:], nu), h_u,
                                                Op.add)
                    else:
                        ck = ut()
                        scal_act(ck[:], k_prev[:], scale=c, chunks=SCHUNK)
                        nc.vector.tensor_tensor(_v3(s_j[:], nu),
                                                _v3(ck[:], nu), h_u, Op.add)
                    m = ut()
                    nc.vector.tensor_tensor(m[:], sw_sb[:], s_j[:], Op.mult)
                    z = ut()
                    nc.vector.tensor_tensor(z[:], z0[:], m[:], Op.add)
                    f = ut()
                    scal_act(f[:], z[:], Act.Sigmoid, chunks=SCHUNK)
                    next_masked()
                    fu = ut()
                    scal_act(fu[:], f[:], bias=u_v, chunks=SCHUNK)
                    p = ut()
                    nc.vector.tensor_tensor(p[:], fu[:], s_j[:], Op.mult)
                    k_j = ut()
                    nc.vector.tensor_tensor(k_j[:], f[:], p[:], Op.subtract)
                    na = ut()
                    if st < 4:
                        k2x = ut()
                        scal_act(k2x[:], k_j[:], scale=2.0, chunks=SCHUNK)
                        nc.vector.tensor_tensor(na[:], k2x[:], acc[:], Op.add)
                    else:
                        nc.vector.tensor_tensor(na[:], acc[:], k_j[:], Op.add)
                    acc = na
                    k_prev = k_j
                # out_u = h + acc/6
                acc6 = ut()
                scal_act(acc6[:], acc[:], scale=DT / 6.0, chunks=SCHUNK)
                nc.vector.tensor_tensor(o_u, _v3(acc6[:], nu), h_u, Op.add)

            while msubs:
                next_masked()

            # ---- out DMA per half group ----
            half = G * N // 2
            nc.gpsimd.dma_start(out_d[:, g * G * N:g * G * N + half],
                              out_t[:, :half])
            nc.gpsimd.dma_start(out_d[:, g * G * N + half:(g + 1) * G * N],
                              out_t[:, half:])

    nc.compile()
    return nc


def kernel(x, h, W_in, w_rec, mask, bias, tau, A, sigma):
    global LAST_EXEC_TIME_NS, LAST_RESULT
    x = np.asarray(x)
    h = np.asarray(h)
    W_in = np.asarray(W_in)
    w_rec = np.asarray(w_rec, dtype=np.float32)
    maskf = np.asarray(mask).astype(np.float32)

    b_v = _uniform(bias, "bias")
    tau_v = _uniform(tau, "tau")
    A_v = _uniform(A, "A")
    sig_v = _uniform(sigma, "sigma")
    u_v = 1.0 / tau_v
    sb_v = sig_v * b_v

    sw = sig_v * w_rec * maskf  # [N]
    unm = np.flatnonzero(sw != 0.0)
    msk = np.flatnonzero(sw == 0.0)
    nu_raw = len(unm)
    nu = min(N, ((nu_raw + 7) // 8) * 8) if nu_raw else 0
    extra = nu - nu_raw
    perm = np.concatenate([unm, msk[:extra], msk[extra:]]).astype(np.int64)
    nm = N - nu

    # masked closed-form cubic R(f) = DT*P(DT*(f+u))/6,
    # P(beta) = -beta^3/4 + beta^2 - 3 beta + 6
    pP = np.poly1d([-0.25, 1.0, -3.0, 6.0])
    comp = pP(np.poly1d([DT, DT * u_v])) * (DT / 6.0)
    rc = np.zeros(4)
    rc[4 - len(comp.coeffs):] = comp.coeffs  # [c3, c2, c1, c0]

    if os.environ.get("BASS_TRACE"):
        _install_ntff_hook()

    nc = _build(nu, nm, sig_v, sb_v, u_v, A_v, rc)

    # ---- host-side marshalling ----
    xT = np.ascontiguousarray(x.T.astype(np.float16))          # [I=128, B]
    Wt = np.ascontiguousarray(W_in[perm].T.astype(np.float16))  # [I=128, N]
    hp = h[:, perm].astype(np.float16)                          # [B, N]
    in_maps = []
    for c in range(N_CORES):
        sl = slice(c * BS, (c + 1) * BS)
        xc = np.ascontiguousarray(xT[:, sl])
        hc = np.ascontiguousarray(
            hp[sl].reshape(T, P, N).transpose(1, 0, 2).reshape(P, T * N))
        im = {"x": xc, "h": hc, "w": Wt}
        if nu:
            swp = np.tile(sw[perm][:nu].astype(np.float16), G)   # [G*nu]
            im["sw"] = np.ascontiguousarray(
                np.broadcast_to(swp, (P, G * nu)))
        in_maps.append(im)

    res = run_bass_kernel_spmd(nc, in_maps, core_ids=list(range(N_CORES)))
    LAST_RESULT = res
    LAST_EXEC_TIME_NS = res.exec_time_ns

    outs = []
    for c in range(N_CORES):
        o = np.asarray(res.results[c]["out"])
        outs.append(o.reshape(P, T, N).transpose(1, 0, 2).reshape(BS, N))
    of = np.concatenate(outs, 0).astype(np.float32)
    out = np.empty_like(of)
    out[:, perm] = of
    return out
